# revision 17
# baseline (speedup 1.0000x reference)
"""Causal self-attention (B=4, L=2048, D=1536, H=24, RoPE) on 8 trn2 NeuronCores.

Sharding: hybrid batch x head-group tensor parallel. Core c handles batch
c//2 with head group c%2 (12 of 24 heads). All matmul operands are bf16
(fp32 PSUM accumulation); intermediates (qkT, v, attn) live entirely in SBUF
-- no DRAM round trips. The output projection is computed per 512-token
block as a bf16 partial over the core's 768 features, and a pairwise
ReduceScatter per block (pipelined with the next block's attention) sums
the two head-groups and scatters 256-token slices to each core.

Device layout (matmuls are lhsT.T @ rhs with contraction on partitions):
- qkT is computed feature-major (S^T layout) so attention scores
  S^T[k,q] = kT.T @ qT need no transposes; the two heads of a pair occupy
  partition halves, so their score matmuls row-tile the PE (K=64 each at
  rows 0/64) and run concurrently.
- RoPE via a block-diagonal rotation matmul (PSUM slot reused) + cos/sin
  table multiplies on the vector engine.
- Softmax on S^T: exp on the scalar engine (scale folded; scores are O(1)),
  causal mask by bf16 multiply after exp, denominator via a ones-column
  appended to V (M=65 PV matmuls), division with reciprocal_approx_fast.
- Output projection contracts the core's 6 feature chunks from SBUF attn.
"""

import sys

sys.path.insert(0, "/opt/trn_rl_repo")

import numpy as np
import ml_dtypes

import concourse.bass as bass
import concourse.mybir as mybir
import concourse.tile as tile
from concourse import bacc
from concourse.bass_utils import run_bass_kernel_spmd

P = 128
B, L, D = 4, 2048, 1536
H, DH = 24, 64
NPAIR = 6            # head pairs per core
KC = D // P          # 12 contraction chunks
NQ = L // 512        # 4 token quarters
TC = L // P          # 16 token chunks of 128
ROPE_BASE = 10000.0

f32 = mybir.dt.float32
bf16 = mybir.dt.bfloat16
BF = ml_dtypes.bfloat16

RG = [[0, 1], [2, 3], [4, 5], [6, 7]]

_CACHE = {}
LAST_RESULT = None
DEBUG_DUMPS = False

# phase-1 chunk order: finish head-pair (m, 6+m) couples early
M_ORDER = [0, 6, 1, 7, 2, 8, 3, 9, 4, 10, 5, 11]


def _build_nc():
    nc = bacc.Bacc(
        "TRN2",
        target_bir_lowering=False,
        debug=False,
        num_devices=8,
    )

    xt_d = nc.dram_tensor("xt", [NQ, P, KC, 512], bf16, kind="ExternalInput")
    wqk_d = nc.dram_tensor("wqk", [12, P, KC, P], bf16, kind="ExternalInput")
    wv_d = nc.dram_tensor("wv", [P, KC, 768], bf16, kind="ExternalInput")
    wo_d = nc.dram_tensor("wo", [P, NPAIR, D], bf16, kind="ExternalInput")
    cos_d = nc.dram_tensor("cos", [P, L], bf16, kind="ExternalInput")
    sin_d = nc.dram_tensor("sin", [P, L], bf16, kind="ExternalInput")
    rot_d = nc.dram_tensor("rot", [P, P], bf16, kind="ExternalInput")
    mask_d = nc.dram_tensor("mask", [P, 4, 512], bf16, kind="ExternalInput")
    out_ext = nc.dram_tensor("out", [4, 256, D], bf16, kind="ExternalOutput")
    rsout_d = nc.dram_tensor("rsout", [4, 256, D], bf16, kind="Internal")
    if DEBUG_DUMPS:
        dbg_qk = nc.dram_tensor("dbg_qk", [2, P, 512], bf16, kind="ExternalOutput")
        dbg_v = nc.dram_tensor("dbg_v", [P, NPAIR, 130], bf16, kind="ExternalOutput")
        dbg_attn = nc.dram_tensor(
            "dbg_attn", [P, NPAIR, 512], bf16, kind="ExternalOutput"
        )
        dbg_pt = nc.dram_tensor("dbg_pt", [P, 2, 512], bf16, kind="ExternalOutput")
        dbg_pso = nc.dram_tensor("dbg_pso", [2, 65, 512], f32, kind="ExternalOutput")
        dbg_rb = nc.dram_tensor("dbg_rb", [2, 64, 512], f32, kind="ExternalOutput")

    Exp = mybir.ActivationFunctionType.Exp

    from contextlib import ExitStack

    with tile.TileContext(nc) as tc:
        with ExitStack() as stack:
            pool = lambda n, **kw: stack.enter_context(tc.tile_pool(name=n, **kw))
            dram = pool("dram", bufs=1, space="DRAM")
            tabs = pool("tabs", bufs=1)
            qkp = pool("qkp", bufs=1)
            vp = pool("vp", bufs=1)
            attnp = pool("attnp", bufs=1)
            psA = pool("psA", bufs=2, space="PSUM")
            psS = pool("psS", bufs=2, space="PSUM")
            psO = pool("psO", bufs=1, space="PSUM")
            ptp = pool("ptp", bufs=3)
            rcp = pool("rcp", bufs=1)
            rbp = pool("rbp", bufs=2)
            t2p = pool("t2p", bufs=3)
            st3p = pool("st3p", bufs=2)
            partial = dram.tile([4, 512, D], bf16)

            cos_sb = tabs.tile([P, L], bf16, name="cos_sb", tag="cos")
            nc.scalar.dma_start(cos_sb[:], cos_d[:])
            sin_sb = tabs.tile([P, L], bf16, name="sin_sb", tag="sin")
            nc.scalar.dma_start(sin_sb[:], sin_d[:])
            rot_sb = tabs.tile([P, P], bf16, name="rot_sb", tag="rot")
            nc.scalar.dma_start(rot_sb[:], rot_d[:])
            mask_sb = tabs.tile([P, 4, 512], bf16, name="mask_sb", tag="mask")
            nc.scalar.dma_start(mask_sb[:], mask_d[:])
            wv_sb = tabs.tile([P, KC, 768], bf16, name="wv_sb", tag="wv")
            nc.scalar.dma_start(wv_sb[:], wv_d[:])
            wo_sb = tabs.tile([P, NPAIR, D], bf16, name="wo_sb", tag="wo")
            nc.scalar.dma_start(wo_sb[:], wo_d[:])

            qk_t = {}
            for m in range(12):
                for q in range(NQ):
                    qk_t[(m, q)] = qkp.tile(
                        [P, 512], bf16, name=f"qk_{m}_{q}", tag=f"qk_{m}_{q}"
                    )
            v_t = [
                vp.tile([P, NPAIR, 130], bf16, name=f"v_{j}", tag=f"v_{j}")
                for j in range(TC)
            ]
            attn_t = [
                attnp.tile([P, NPAIR, 512], bf16, name=f"attn_{i}", tag=f"attn_{i}")
                for i in range(NQ)
            ]

            # ones columns for the PV denominator rows
            for j in range(TC):
                nc.vector.memset(v_t[j][:, :, 64:65], 1.0)
                nc.vector.memset(v_t[j][:, :, 129:130], 1.0)

            # ---------------- Phase 1: projections + RoPE ----------------
            with ExitStack() as p1stack:
                p1pool = lambda n, **kw: p1stack.enter_context(
                    tc.tile_pool(name=n, **kw)
                )
                xtp = p1pool("xtp", bufs=2)
                wqkp = p1pool("wqkp", bufs=2)
                stp = p1pool("stp", bufs=2)
                tmp = p1pool("tmp", bufs=2)
                for q in range(NQ):
                    qsl = slice(q * 512, (q + 1) * 512)
                    xtq = xtp.tile([P, KC, 512], bf16, tag="xtq", name=f"xtq{q}")
                    nc.sync.dma_start(xtq[:], xt_d[q])

                    for m in M_ORDER:
                        w_t = wqkp.tile([P, KC, P], bf16, tag="wqk", name=f"w{q}_{m}")
                        nc.sync.dma_start(w_t[:], wqk_d[m])
                        ps = psA.tile([P, 512], f32, tag="ps", name=f"ps{q}_{m}")
                        for k in range(KC):
                            nc.tensor.matmul(
                                ps[:],
                                w_t[:, k, :],
                                xtq[:, k, :],
                                start=(k == 0),
                                stop=(k == KC - 1),
                            )
                        st = stp.tile([P, 512], bf16, tag="st", name=f"st{q}_{m}")
                        nc.scalar.copy(st[:], ps[:])
                        # rotation matmul reuses the same PSUM slot
                        nc.tensor.matmul(ps[:], rot_sb[:], st[:], start=True, stop=True)
                        qk = qk_t[(m, q)]
                        nc.vector.tensor_mul(qk[:], st[:], cos_sb[:, qsl])
                        tm = tmp.tile([P, 512], bf16, tag="tm", name=f"tm{q}_{m}")
                        nc.vector.tensor_mul(tm[:], ps[:], sin_sb[:, qsl])
                        nc.vector.tensor_add(qk[:], qk[:], tm[:])

                    # V projection for this quarter's 4 token chunks
                    for mtl in range(4):
                        mt = 4 * q + mtl
                        csl = slice(mtl * 128, (mtl + 1) * 128)
                        for half in range(2):
                            fsl = slice(half * 384, (half + 1) * 384)
                            psv = psA.tile(
                                [P, 512], f32, tag="ps", name=f"psv{mt}_{half}"
                            )
                            for k in range(KC):
                                nc.tensor.matmul(
                                    psv[:, :384],
                                    xtq[:, k, csl],
                                    wv_sb[:, k, fsl],
                                    start=(k == 0),
                                    stop=(k == KC - 1),
                                )
                            for hl in range(6):
                                h = half * 6 + hl
                                hp, hh = h // 2, h % 2
                                nc.vector.tensor_copy(
                                    v_t[mt][:, hp, 65 * hh : 65 * hh + 64],
                                    psv[:, hl * 64 : (hl + 1) * 64],
                                )

            # ---------------- Phase 2+3 interleaved per token quarter ----
            for i in range(NQ):
                njb = 4 * i + 4
                for hp in range(NPAIR):
                    qt = qk_t[(hp, i)]
                    pso = [
                        psO.tile([P, 512], f32, tag=f"pso{hh}", name=f"pso{i}_{hp}_{hh}")
                        for hh in range(2)
                    ]
                    pts = [None] * njb

                    def emit_pv(j, is_last):
                        for hh in range(2):
                            nc.tensor.matmul(
                                pso[hh][0:65, :],
                                v_t[j][:, hp, 65 * hh : 65 * hh + 65],
                                pts[j][:, hh, :],
                                start=(j == 0),
                                stop=is_last,
                            )

                    for j in range(njb):
                        jq, jr = divmod(j, 4)
                        kt = qk_t[(6 + hp, jq)]
                        jsl = slice(jr * 128, (jr + 1) * 128)
                        pss = psS.tile(
                            [P, 2, 512], f32, tag="pss", name=f"pss{i}_{hp}_{j}"
                        )
                        for hh in range(2):
                            o = 64 * hh
                            nc.tensor.matmul(
                                pss[:, hh, :],
                                kt[o : o + 64, jsl],
                                qt[o : o + 64, :],
                                start=True,
                                stop=True,
                            )
                        pt = ptp.tile([P, 2, 512], bf16, tag="pt", name=f"pt{i}_{hp}_{j}")
                        nc.scalar.activation(
                            pt.rearrange("p a b -> p (a b)"),
                            pss.rearrange("p a b -> p (a b)"),
                            Exp,
                            scale=0.125,
                        )
                        g = j - 4 * i
                        if g >= 0:
                            for hh in range(2):
                                nc.vector.tensor_mul(
                                    pt[:, hh, :], pt[:, hh, :], mask_sb[:, g, :]
                                )
                        pts[j] = pt
                        if DEBUG_DUMPS and i == 0 and hp == 0 and j == 0:
                            nc.sync.dma_start(dbg_pt[:], pt[:])
                        if j >= 1:
                            emit_pv(j - 1, is_last=False)
                    emit_pv(njb - 1, is_last=True)

                    if DEBUG_DUMPS and i == 0 and hp == 0:
                        for hh in range(2):
                            dstg = st3p.tile(
                                [65, 512], f32, tag="dbgpso", name=f"dbgpso{hh}", bufs=1
                            )
                            nc.vector.tensor_copy(dstg[:], pso[hh][0:65, :])
                            nc.sync.dma_start(dbg_pso[hh], dstg[:])

                    for hh in range(2):
                        dn = rcp.tile([1, 512], f32, tag="dn", name=f"dn{i}_{hp}_{hh}")
                        nc.vector.tensor_copy(dn[:], pso[hh][64:65, :])
                        rc = rcp.tile([1, 512], f32, tag="rc", name=f"rc{i}_{hp}_{hh}")
                        nc.vector.reciprocal_approx_fast(rc[:], dn[:])
                        rb = rbp.tile([64, 512], f32, tag="rb", name=f"rb{i}_{hp}_{hh}")
                        nc.gpsimd.partition_broadcast(rb[:], rc[:])
                        if DEBUG_DUMPS and i == 0 and hp == 0:
                            nc.sync.dma_start(dbg_rb[hh], rb[:])
                        t2 = t2p.tile([64, 512], bf16, tag="t2", name=f"t2{i}_{hp}_{hh}")
                        nc.vector.tensor_mul(t2[:], pso[hh][0:64, :], rb[:])
                        nc.sync.dma_start(
                            attn_t[i][64 * hh : 64 * hh + 64, hp, :], t2[:]
                        )

                # ---- output projection for this token block ----
                for mtl in range(4):
                    msl = slice(mtl * 128, (mtl + 1) * 128)
                    st3 = st3p.tile([P, D], bf16, tag="st3", name=f"st3_{i}_{mtl}")
                    for et in range(3):
                        esl = slice(et * 512, (et + 1) * 512)
                        ps3 = psA.tile([P, 512], f32, tag="ps", name=f"ps3_{i}_{mtl}_{et}")
                        for hp in range(NPAIR):
                            nc.tensor.matmul(
                                ps3[:],
                                attn_t[i][:, hp, msl],
                                wo_sb[:, hp, esl],
                                start=(hp == 0),
                                stop=(hp == NPAIR - 1),
                            )
                        nc.scalar.copy(st3[:, esl], ps3[:])
                    nc.sync.dma_start(partial[i, msl, :], st3[:])

                nc.gpsimd.collective_compute(
                    "ReduceScatter",
                    mybir.AluOpType.add,
                    replica_groups=RG,
                    ins=[partial[i].opt()],
                    outs=[rsout_d[i].opt()],
                )

            # out copies at the end: an out-DMA waiting on RS_i must never sit
            # in the sync FIFO in front of attention/t2 DMAs (strict per-engine
            # FIFO would head-of-line block them for the collective's duration)
            for i in range(NQ):
                nc.sync.dma_start(out_ext[i], rsout_d[i])

            if DEBUG_DUMPS:
                nc.sync.dma_start(dbg_qk[0], qk_t[(0, 0)][:])
                nc.sync.dma_start(dbg_qk[1], qk_t[(6, 0)][:])
                nc.sync.dma_start(dbg_v[:], v_t[0][:])
                nc.sync.dma_start(dbg_attn[:], attn_t[0][:])

    nc.compile()
    return nc


def _rope_tables(pos_offset):
    inv_freq = 1.0 / (ROPE_BASE ** (np.arange(0, DH, 2, dtype=np.float32) / DH))
    t = np.arange(L, dtype=np.float32) + np.float32(pos_offset)
    freqs = np.outer(t, inv_freq)                      # (L, 32)
    emb = np.concatenate([freqs, freqs], axis=-1)      # (L, 64)
    cosT = np.cos(emb).T.astype(np.float32)            # (64, L)
    sinT = np.sin(emb).T.astype(np.float32)
    cos2 = np.concatenate([cosT, cosT], axis=0)        # (128, L)
    sin2 = np.concatenate([sinT, sinT], axis=0)
    return (
        np.ascontiguousarray(cos2).astype(BF),
        np.ascontiguousarray(sin2).astype(BF),
    )


def _rot_matrix():
    R = np.zeros((DH, DH), dtype=np.float32)
    R[:32, 32:] = -np.eye(32, dtype=np.float32)
    R[32:, :32] = np.eye(32, dtype=np.float32)
    R2 = np.zeros((P, P), dtype=np.float32)
    R2[:64, :64] = R
    R2[64:, 64:] = R
    return np.ascontiguousarray(R2.T).astype(BF)


def _masks():
    m = np.zeros((4, P, 512), dtype=np.float32)
    kr = np.arange(P)[:, None]
    c = np.arange(512)[None, :]
    for g in range(4):
        m[g] = (P * g + kr <= c).astype(np.float32)
    return np.ascontiguousarray(np.transpose(m, (1, 0, 2))).astype(BF)  # (128,4,512)


def _make_in_maps(x, w_qkv, w_out, pos_offset):
    x = np.asarray(x, dtype=np.float32)
    w_qkv = np.asarray(w_qkv, dtype=np.float32)
    w_out = np.asarray(w_out, dtype=np.float32)

    cos2, sin2 = _rope_tables(int(pos_offset))
    rotT = _rot_matrix()
    maskT = _masks()

    in_maps = []
    for c in range(8):
        b, g = c // 2, c % 2
        QF = 768
        rows_q = slice(g * QF, (g + 1) * QF)
        rows_k = slice(D + g * QF, D + (g + 1) * QF)
        rows_v = slice(2 * D + g * QF, 2 * D + (g + 1) * QF)

        # xt: [4 quarters, 128 p, 12 kc, 512 t], per-partition contiguous
        xT = np.ascontiguousarray(x[b].T)                        # (1536, 2048)
        xt = np.ascontiguousarray(
            xT.reshape(KC, P, NQ, 512).transpose(2, 1, 0, 3)
        ).astype(BF)

        # wqk: [12 m, 128 p, 12 kc, 128 f]
        wqkT = np.concatenate([w_qkv[rows_q], w_qkv[rows_k]], axis=0).T  # (1536,1536)
        wqk = np.ascontiguousarray(
            wqkT.reshape(KC, P, 12, P).transpose(2, 1, 0, 3)
        ).astype(BF)

        # wv: [128 p, 12 kc, 768 f]
        wvT = w_qkv[rows_v].T                                    # (1536, 768)
        wv = np.ascontiguousarray(wvT.reshape(KC, P, QF).transpose(1, 0, 2)).astype(BF)

        # wo: [128 p, 6 c, 1536 e]
        woT = w_out[:, g * QF : (g + 1) * QF].T                  # (768, 1536)
        wo = np.ascontiguousarray(woT.reshape(NPAIR, P, D).transpose(1, 0, 2)).astype(
            BF
        )

        in_maps.append(
            {
                "xt": xt,
                "wqk": wqk,
                "wv": wv,
                "wo": wo,
                "cos": cos2,
                "sin": sin2,
                "rot": rotT,
                "mask": maskT,
            }
        )
    return in_maps


def _assemble(results):
    out = np.empty((B, L, D), dtype=np.float32)
    for c in range(8):
        b, g = c // 2, c % 2
        blk = np.asarray(results[c]["out"], dtype=np.float32)    # (4, 256, 1536)
        for i in range(4):
            r0 = i * 512 + g * 256
            out[b, r0 : r0 + 256] = blk[i]
    return out


def kernel(x, w_qkv, w_out, pos_offset):
    global LAST_RESULT
    if "nc" not in _CACHE:
        _CACHE["nc"] = _build_nc()
    nc = _CACHE["nc"]
    in_maps = _make_in_maps(x, w_qkv, w_out, pos_offset)
    res = run_bass_kernel_spmd(nc, in_maps, list(range(8)))
    LAST_RESULT = res
    return _assemble(res.results)
